# revision 1
# baseline (speedup 1.0000x reference)
"""GAT (2-layer graph attention) Trainium2 kernel, 8-core SPMD.

Strategy:
  - Host: degree-balanced node permutation (snake over degree sort), nodes
    padded to 8*SHARD; per-core slot tables (node-tile x slot -> src id) with
    -1e30 masks for dummy slots pointing at an all-zero pad node.
  - Device (identical SPMD program, per-core data):
      * node phase (replicated): h held transposed [D, N]; per 128-node tile
        matmul(lhsT=hT_tile, rhs=Wkv[128,256]) -> row-layout k|v table in HBM.
        q/ws computed only for the core's own shard (partition-id dynamic DMA).
      * edge phase (sharded by dst): nodes-on-partitions; per tile indirect-DMA
        gather of k|v rows by slot src; scores via DVE mul+reduce; segment
        softmax = free-axis reductions; exp on ScalarE with per-partition bias
        and accumulated denominator; attn-weighted v accumulated over slots via
        identity matmuls into PSUM.
      * one AllGather of h1T shards between the two layers; fc1/gelu/fc2 on the
        local shard; host inverse-permutes the [4, SHARD] shards.
"""
import numpy as np
from contextlib import ExitStack

import concourse.bacc as bacc
import concourse.bass as bass
import concourse.tile as tile
from concourse import mybir
from concourse.bass_utils import run_bass_kernel_spmd
from concourse.masks import make_identity

P = 128
NEG = -1.0e30
F32 = mybir.dt.float32
BF16 = mybir.dt.bfloat16
I32 = mybir.dt.int32
AF = mybir.ActivationFunctionType
ALU = mybir.AluOpType
AX = mybir.AxisListType

FULL_CFG = dict(
    N=100000, E=1600000, IN_DIM=16, D=128, H=2, FC=128, OUT=4, L=2,
    NCORES=8, SHARD=12800,
)


def _derive(cfg):
    c = dict(cfg)
    c["DK"] = c["D"] // c["H"]
    c["N_PAD"] = c["NCORES"] * c["SHARD"]
    c["NTILES"] = c["SHARD"] // P
    assert c["SHARD"] % 512 == 0
    c["NRUNS"] = c["SHARD"] // 512          # 512-col runs per shard
    c["NBLK"] = c["N_PAD"] // 512           # 512-col runs over all nodes
    return c


def preprocess(cfg, src, dst):
    c = _derive(cfg)
    N, NCORES, SHARD, N_PAD, NTILES = (
        c["N"], c["NCORES"], c["SHARD"], c["N_PAD"], c["NTILES"])
    E = len(src)
    deg = np.bincount(dst, minlength=N)
    order = np.argsort(deg, kind="stable")
    perm = np.empty(N, dtype=np.int64)
    perm[order] = (np.arange(N) % NCORES) * SHARD + (np.arange(N) // NCORES)

    nsrc = perm[src]
    ndst = perm[dst]
    ndeg = np.bincount(ndst, minlength=N_PAD)

    K_t = np.zeros(NTILES, dtype=np.int64)
    for t in range(NTILES):
        mx = 0
        for cc in range(NCORES):
            b = cc * SHARD + t * P
            mx = max(mx, int(ndeg[b:b + P].max()))
        K_t[t] = max(mx, 1)
    tile_off = np.concatenate([[0], np.cumsum(K_t)[:-1]])
    S_TOT = int(K_t.sum())
    DUMMY = N_PAD - 1

    # idx/mask in [128, S_TOT] partition-major layout
    idxs = np.full((NCORES, P, S_TOT), DUMMY, dtype=np.int32)
    masks = np.full((NCORES, P, S_TOT), np.float32(NEG), dtype=np.float32)

    eorder = np.argsort(ndst, kind="stable")
    sdst = ndst[eorder]
    ssrc = nsrc[eorder]
    starts = np.zeros(N_PAD, dtype=np.int64)
    np.cumsum(ndeg[:-1], out=starts[1:])
    rank = np.arange(E) - starts[sdst]
    c_of = sdst // SHARD
    within = sdst % SHARD
    t_of = within // P
    p_of = within % P
    scol = tile_off[t_of] + rank
    idxs[c_of, p_of, scol] = ssrc
    masks[c_of, p_of, scol] = 0.0

    return dict(perm=perm, K_t=K_t, tile_off=tile_off, S_TOT=S_TOT,
                idxs=idxs, masks=masks, cfg=c)


def build_program(c, K_t, tile_off, S_TOT):
    NCORES, SHARD, N_PAD, NTILES = c["NCORES"], c["SHARD"], c["N_PAD"], c["NTILES"]
    IN_DIM, D, H, DK, L = c["IN_DIM"], c["D"], c["H"], c["DK"], c["L"]
    FC, OUT = c["FC"], c["OUT"]
    NRUNS, NBLK = c["NRUNS"], c["NBLK"]
    K_MAX = int(max(K_t))
    CH = 8  # slots per DVE chunk

    nc = bacc.Bacc("TRN2", target_bir_lowering=False, debug=False,
                   num_devices=NCORES)

    xT_d = nc.dram_tensor("xT", [IN_DIM, N_PAD], F32, kind="ExternalInput")
    wfc0_d = nc.dram_tensor("wfc0", [IN_DIM, D], F32, kind="ExternalInput")
    wkv_d = nc.dram_tensor("wkv", [L, D, 2 * D], F32, kind="ExternalInput")
    wq_d = nc.dram_tensor("wq", [L, D, D], F32, kind="ExternalInput")
    ws_d = nc.dram_tensor("ws", [L, D, D], F32, kind="ExternalInput")
    wfc1_d = nc.dram_tensor("wfc1", [D, FC], F32, kind="ExternalInput")
    wfc2_d = nc.dram_tensor("wfc2", [FC, OUT], F32, kind="ExternalInput")
    idxs_d = nc.dram_tensor("idxs", [P, S_TOT], I32, kind="ExternalInput")
    masks_d = nc.dram_tensor("masks", [P, S_TOT], F32, kind="ExternalInput")

    h0T_d = nc.dram_tensor("h0T", [D, N_PAD], F32)
    kvtab_d = nc.dram_tensor("kvtab", [N_PAD, 2 * D], F32)
    qtab_d = nc.dram_tensor("qtab", [SHARD, D], F32)
    wstab_d = nc.dram_tensor("wstab", [SHARD, D], F32)
    ccin_d = nc.dram_tensor("ccin", [D, SHARD], F32)
    ccout_d = nc.dram_tensor("ccout", [NCORES, D, SHARD], F32, addr_space="Shared")
    h2T_d = nc.dram_tensor("h2T", [D, SHARD], F32)
    outT_d = nc.dram_tensor("outT", [OUT, SHARD], F32, kind="ExternalOutput")

    # [128, 8, SHARD] block view of h0T for partition-id dynamic reads
    h0T_v = h0T_d[:, :].rearrange("d (b s) -> d b s", b=NCORES)

    with tile.TileContext(nc) as tc, ExitStack() as ctx:
        cp = ctx.enter_context(tc.tile_pool(name="const", bufs=1))
        hp = ctx.enter_context(tc.tile_pool(name="hrun", bufs=3))
        pcp = ctx.enter_context(tc.tile_pool(name="pcopy", bufs=3))
        kvp = ctx.enter_context(tc.tile_pool(name="kv", bufs=2))
        edp = ctx.enter_context(tc.tile_pool(name="edge", bufs=2))
        prp = ctx.enter_context(tc.tile_pool(name="scratch", bufs=2))
        flp = ctx.enter_context(tc.tile_pool(name="fcs", bufs=3))
        pp_ps = ctx.enter_context(tc.tile_pool(name="projps", bufs=2, space="PSUM"))
        res_ps = ctx.enter_context(tc.tile_pool(name="resps", bufs=2, space="PSUM"))
        tr_ps = ctx.enter_context(tc.tile_pool(name="trps", bufs=2, space="PSUM"))
        fc_ps = ctx.enter_context(tc.tile_pool(name="fcps", bufs=2, space="PSUM"))

        # ---- constants ----
        ident = cp.tile([P, P], F32)
        make_identity(nc, ident[:])
        identb = cp.tile([P, P], BF16)
        nc.vector.tensor_copy(out=identb[:], in_=ident[:])
        fc0_sb = cp.tile([IN_DIM, D], F32)
        nc.sync.dma_start(out=fc0_sb[:], in_=wfc0_d[:, :])
        wkv_sb = []
        wq_sb = []
        ws_sb = []
        for l in range(L):
            t1 = cp.tile([D, 2 * D], F32, tag=f"wkv{l}")
            nc.sync.dma_start(out=t1[:], in_=wkv_d[l, :, :])
            wkv_sb.append(t1)
            t2 = cp.tile([D, D], F32, tag=f"wq{l}")
            nc.sync.dma_start(out=t2[:], in_=wq_d[l, :, :])
            wq_sb.append(t2)
            t3 = cp.tile([D, D], F32, tag=f"ws{l}")
            nc.sync.dma_start(out=t3[:], in_=ws_d[l, :, :])
            ws_sb.append(t3)
        fc1_sb = cp.tile([D, FC], F32)
        nc.sync.dma_start(out=fc1_sb[:], in_=wfc1_d[:, :])
        fc2_sb = cp.tile([FC, OUT], F32)
        nc.sync.dma_start(out=fc2_sb[:], in_=wfc2_d[:, :])
        idx_sb = cp.tile([P, S_TOT], I32)
        nc.sync.dma_start(out=idx_sb[:], in_=idxs_d[:, :])
        mask_sb = cp.tile([P, S_TOT], F32)
        nc.sync.dma_start(out=mask_sb[:], in_=masks_d[:, :])

        cid = nc.sync.partition_id()

        # pre-zero the kv gather slots (avoid NaN garbage in dummy lanes)
        z0 = kvp.tile([P, K_MAX, 2 * D], F32, tag="kv")
        nc.vector.memset(z0[:], 0.0)
        z1 = kvp.tile([P, K_MAX, 2 * D], F32, tag="kv")
        nc.vector.memset(z1[:], 0.0)

        def emit_kv_block(layer, b, hb):
            """kv projections for the 4 tiles of 512-col block b, batched write."""
            stage = pcp.tile([P, 4, 2 * D], F32, tag="kstage")
            for j in range(4):
                kps = pp_ps.tile([P, 2 * D], F32, space="PSUM", tag="pp")
                nc.tensor.matmul(out=kps[:], lhsT=hb[:, j * P:(j + 1) * P],
                                 rhs=wkv_sb[layer][:], start=True, stop=True)
                if j % 2 == 0:
                    nc.scalar.activation(out=stage[:, j, :], in_=kps[:],
                                         func=AF.Copy)
                else:
                    nc.vector.tensor_copy(out=stage[:, j, :], in_=kps[:])
            nc.sync.dma_start(
                out=kvtab_d[b * 512:(b + 1) * 512, :].rearrange(
                    "(j p) c -> p j c", p=P),
                in_=stage[:])

        # ---- h0 = fc0(x) fused with layer-0 kv projections ----
        for b in range(NBLK):
            xb = hp.tile([IN_DIM, 512], F32, tag="xblk")
            nc.sync.dma_start(out=xb[:], in_=xT_d[:, b * 512:(b + 1) * 512])
            ps = fc_ps.tile([P, 512], F32, space="PSUM", tag="fc")
            nc.tensor.matmul(out=ps[:], lhsT=fc0_sb[:], rhs=xb[:],
                             start=True, stop=True)
            hs = hp.tile([P, 512], F32, tag="hcp")
            nc.scalar.activation(out=hs[:], in_=ps[:], func=AF.Copy)
            nc.sync.dma_start(out=h0T_d[:, b * 512:(b + 1) * 512], in_=hs[:])
            emit_kv_block(0, b, hs)

        for layer in range(L):
            # ---- kv projections over ALL node tiles (layer 0 done above) ----
            if layer > 0:
                for b in range(NBLK):
                    hb = hp.tile([P, 512], F32, tag="hrun")
                    cb = b // NRUNS
                    co = (b % NRUNS) * 512
                    nc.sync.dma_start(out=hb[:],
                                      in_=ccout_d[cb, :, co:co + 512])
                    emit_kv_block(layer, b, hb)

            # ---- q/ws projections for own shard (dynamic core offset) ----
            for r in range(NRUNS):
                hb = hp.tile([P, 512], F32, tag="hrun")
                co = r * 512
                if layer == 0:
                    nc.sync.dma_start(
                        out=hb[:],
                        in_=h0T_v[:, bass.ds(cid, 1), co:co + 512].squeeze(1))
                else:
                    nc.sync.dma_start(
                        out=hb[:],
                        in_=ccout_d[bass.ds(cid, 1), :, co:co + 512].squeeze(0))
                qstage = pcp.tile([P, 4, D], F32, tag="qstage")
                wstage = pcp.tile([P, 4, D], F32, tag="wstage")
                for j in range(4):
                    qps = pp_ps.tile([P, 2 * D], F32, space="PSUM", tag="pp")
                    nc.tensor.matmul(out=qps[:, :D], lhsT=hb[:, j * P:(j + 1) * P],
                                     rhs=wq_sb[layer][:], start=True, stop=True)
                    if j % 2 == 0:
                        nc.scalar.activation(out=qstage[:, j, :], in_=qps[:, :D],
                                             func=AF.Copy)
                    else:
                        nc.vector.tensor_copy(out=qstage[:, j, :], in_=qps[:, :D])
                    sps = pp_ps.tile([P, 2 * D], F32, space="PSUM", tag="pp")
                    nc.tensor.matmul(out=sps[:, :D], lhsT=hb[:, j * P:(j + 1) * P],
                                     rhs=ws_sb[layer][:], start=True, stop=True)
                    if j % 2 == 1:
                        nc.scalar.activation(out=wstage[:, j, :], in_=sps[:, :D],
                                             func=AF.Copy)
                    else:
                        nc.vector.tensor_copy(out=wstage[:, j, :], in_=sps[:, :D])
                nc.sync.dma_start(
                    out=qtab_d[r * 512:(r + 1) * 512, :].rearrange(
                        "(j p) c -> p j c", p=P),
                    in_=qstage[:])
                nc.sync.dma_start(
                    out=wstab_d[r * 512:(r + 1) * 512, :].rearrange(
                        "(j p) c -> p j c", p=P),
                    in_=wstage[:])

            # ---- edge phase, per node tile ----
            for t in range(NTILES):
                off = int(tile_off[t])
                K = int(K_t[t])
                if t % 4 == 0:
                    qg = edp.tile([P, 4, D], F32, tag="qg")
                    nc.sync.dma_start(
                        out=qg[:],
                        in_=qtab_d[t * P:(t + 4) * P, :].rearrange(
                            "(j p) c -> p j c", p=P))
                    wsg = edp.tile([P, 4, D], F32, tag="wsg")
                    nc.sync.dma_start(
                        out=wsg[:],
                        in_=wstab_d[t * P:(t + 4) * P, :].rearrange(
                            "(j p) c -> p j c", p=P))
                q_t = qg[:, t % 4]
                kv_t = kvp.tile([P, K_MAX, 2 * D], F32, tag="kv")
                for k in range(K):
                    nc.gpsimd.indirect_dma_start(
                        out=kv_t[:, k, :], out_offset=None,
                        in_=kvtab_d[:, :],
                        in_offset=bass.IndirectOffsetOnAxis(
                            ap=idx_sb[:, off + k:off + k + 1], axis=0),
                    )
                s_t = edp.tile([P, H, K_MAX], F32, tag="s")
                for c0 in range(0, K, CH):
                    cc = min(CH, K - c0)
                    pr = prp.tile([P, CH, D], F32, tag="prod")
                    qb = (q_t[:].rearrange("p (h d) -> p h d", h=H)
                          .unsqueeze(1).to_broadcast([P, cc, H, DK]))
                    nc.vector.tensor_tensor(
                        out=pr[:, :cc].rearrange("p k (h d) -> p k h d", h=H),
                        in0=kv_t[:, c0:c0 + cc, 0:D].rearrange(
                            "p k (h d) -> p k h d", h=H),
                        in1=qb, op=ALU.mult)
                    nc.vector.reduce_sum(
                        out=s_t[:, :, c0:c0 + cc].transpose([0, 2, 1]),
                        in_=pr[:, :cc].rearrange("p k (h d) -> p k h d", h=H),
                        axis=AX.X)
                mb = (mask_sb[:, off:off + K].unsqueeze(1)
                      .to_broadcast([P, H, K]))
                nc.vector.tensor_tensor(out=s_t[:, :, :K], in0=s_t[:, :, :K],
                                        in1=mb, op=ALU.add)
                nsmax = edp.tile([P, H], F32, tag="nsmax")
                nc.vector.tensor_reduce(out=nsmax[:], in_=s_t[:, :, :K],
                                        op=ALU.max, axis=AX.X, negate=True)
                exw = edp.tile([P, H, K_MAX], F32, tag="exw")
                denom = edp.tile([P, H], F32, tag="denom")
                for h in range(H):
                    nc.scalar.activation(
                        out=exw[:, h, :K], in_=s_t[:, h, :K], func=AF.Exp,
                        bias=nsmax[:, h:h + 1], scale=1.0,
                        accum_out=denom[:, h:h + 1])
                rden = edp.tile([P, H], F32, tag="rden")
                nc.vector.reciprocal(out=rden[:], in_=denom[:])
                rb = rden[:].unsqueeze(2).to_broadcast([P, H, K])
                nc.vector.tensor_tensor(out=exw[:, :, :K], in0=exw[:, :, :K],
                                        in1=rb, op=ALU.mult)
                acc = res_ps.tile([P, D], F32, space="PSUM", tag="acc")
                for c0 in range(0, K, CH):
                    cc = min(CH, K - c0)
                    wv = prp.tile([P, CH, D], F32, tag="wv")
                    ab = (exw[:, :, c0:c0 + cc].transpose([0, 2, 1])
                          .unsqueeze(3).to_broadcast([P, cc, H, DK]))
                    nc.vector.tensor_tensor(
                        out=wv[:, :cc].rearrange("p k (h d) -> p k h d", h=H),
                        in0=kv_t[:, c0:c0 + cc, D:2 * D].rearrange(
                            "p k (h d) -> p k h d", h=H),
                        in1=ab, op=ALU.mult)
                    for k in range(cc):
                        nc.tensor.matmul(out=acc[:], lhsT=ident[:],
                                         rhs=wv[:, k, :],
                                         start=(c0 + k == 0),
                                         stop=(c0 + k == K - 1))
                # tail: h_next tile = (wsh + res), gelu (layer 0), transpose
                wsh_t = edp.tile([P, D], F32, tag="wsh")
                nc.sync.dma_start(out=wsh_t[:], in_=wstab_d[t * P:(t + 1) * P, :])
                hn = edp.tile([P, D], F32, tag="hn")
                nc.vector.tensor_tensor(out=hn[:], in0=acc[:], in1=wsh_t[:],
                                        op=ALU.add)
                if layer == 0:
                    hng = edp.tile([P, D], F32, tag="hng")
                    nc.scalar.activation(out=hng[:], in_=hn[:],
                                         func=AF.Gelu_apprx_tanh)
                    hn = hng
                trp = tr_ps.tile([P, D], F32, space="PSUM", tag="tr")
                nc.tensor.transpose(out=trp[:], in_=hn[:], identity=ident[:])
                hT_t = edp.tile([P, D], F32, tag="hTt")
                nc.scalar.activation(out=hT_t[:], in_=trp[:], func=AF.Copy)
                dst = ccin_d if layer == 0 else h2T_d
                nc.sync.dma_start(out=dst[:, t * P:(t + 1) * P], in_=hT_t[:])

            if layer == 0:
                nc.gpsimd.collective_compute(
                    "AllGather", ALU.bypass,
                    replica_groups=[list(range(NCORES))],
                    ins=[ccin_d[:]], outs=[ccout_d[:]],
                )

        # ---- fc1 -> gelu -> fc2 on own shard ----
        for r in range(NRUNS):
            co = r * 512
            hb = hp.tile([P, 512], F32, tag="hrun")
            nc.sync.dma_start(out=hb[:], in_=h2T_d[:, co:co + 512])
            f1 = fc_ps.tile([P, 512], F32, space="PSUM", tag="fc")
            nc.tensor.matmul(out=f1[:], lhsT=fc1_sb[:], rhs=hb[:],
                             start=True, stop=True)
            fa = flp.tile([P, 512], F32, tag="fca")
            nc.scalar.activation(out=fa[:], in_=f1[:], func=AF.Gelu_apprx_tanh)
            f2 = fc_ps.tile([P, 512], F32, space="PSUM", tag="fc")
            nc.tensor.matmul(out=f2[:OUT, :], lhsT=fc2_sb[:], rhs=fa[:],
                             start=True, stop=True)
            oc = flp.tile([OUT, 512], F32, tag="oc")
            nc.scalar.activation(out=oc[:], in_=f2[:OUT, :], func=AF.Copy)
            nc.sync.dma_start(out=outT_d[:, co:co + 512], in_=oc[:])

    nc.compile()
    return nc


def run(inputs, cfg=FULL_CFG, trace=False):
    c = _derive(cfg)
    x = np.asarray(inputs["x"], dtype=np.float32)
    src = np.asarray(inputs["src"])
    dst = np.asarray(inputs["dst"])
    pp = preprocess(cfg, src, dst)
    c = pp["cfg"]

    # padded, permuted, transposed x
    xP = np.zeros((c["N_PAD"], c["IN_DIM"]), np.float32)
    xP[pp["perm"]] = x
    xT = np.ascontiguousarray(xP.T)

    wkv = np.concatenate(
        [np.asarray(inputs["Wk"]), np.asarray(inputs["Wv"])], axis=2
    ).astype(np.float32)  # [L, D, 2D]

    nc = build_program(c, pp["K_t"], pp["tile_off"], pp["S_TOT"])

    in_maps = []
    for cc in range(c["NCORES"]):
        in_maps.append({
            "xT": xT,
            "wfc0": np.asarray(inputs["fc0_w"], np.float32),
            "wkv": wkv,
            "wq": np.asarray(inputs["Wq"], np.float32),
            "ws": np.asarray(inputs["ws_w"], np.float32),
            "wfc1": np.asarray(inputs["fc1_w"], np.float32),
            "wfc2": np.asarray(inputs["fc2_w"], np.float32),
            "idxs": pp["idxs"][cc],
            "masks": pp["masks"][cc],
        })
    r = run_bass_kernel_spmd(nc, in_maps, core_ids=list(range(c["NCORES"])),
                             trace=trace)
    run.last_exec_ns = r.exec_time_ns

    outT = np.stack([r.results[cc]["outT"] for cc in range(c["NCORES"])])
    # node j (original) -> core perm[j]//SHARD, col perm[j]%SHARD
    perm = pp["perm"]
    out = outT[perm // c["SHARD"], :, perm % c["SHARD"]]  # [N, OUT]
    return np.ascontiguousarray(out.astype(np.float32))


def kernel(**inputs):
    return run(inputs, FULL_CFG)

